# revision 6
# baseline (speedup 1.0000x reference)
"""Trainium2 Bass kernel for nn_DALayer (moe_routing, squeeze-excite style).

Computation (per sample b):
    y    = mean(x[b], axis=(H,W))                 # [C]
    h    = relu(W1[dataset[b]] @ y)               # [HID]
    gate = sigmoid(W2[dataset[b]] @ h)            # [C]
    out[b] = x[b] * gate[:, None, None]

Sharding: pure data parallel over batch across 8 NeuronCores (8 samples
per core); expert weights replicated.  Single pass over x: each sample's
x tiles stay resident in SBUF between the mean-reduce and the gate
multiply, so HBM traffic per core is 64 MiB in + 64 MiB out.

Expert routing is done on-device: all three experts' W1 rows are stacked
([96, C]) so one accumulating matmul chain produces h for every expert;
a one-hot mask (built from `dataset` with is_equal compares and a tiny
matmul against a constant block-indicator) zeroes the two unselected
experts' h, and a stacked-W2 matmul then yields the selected expert's
gate directly.
"""

import numpy as np
from contextlib import ExitStack

import concourse.tile as tile
from concourse import bacc, mybir
from concourse import bass_utils

# Problem shapes (hardcoded per contract).
B, C, H, W = 64, 512, 64, 64
HW = H * W                 # 4096 spatial elements
N_CORES = 8
BL = B // N_CORES          # 8 samples per core
NE, HID = 3, 32
M96 = NE * HID             # 96 stacked expert-hidden rows
P = 128                    # SBUF partitions
J = C // P                 # 4 channel chunks of 128

_nc_cache = None


def _build():
    """Build + compile the per-core Bass module (cached)."""
    global _nc_cache
    if _nc_cache is not None:
        return _nc_cache

    f32 = mybir.dt.float32
    i32 = mybir.dt.int32
    FT = mybir.ActivationFunctionType

    nc = bacc.Bacc(
        "TRN2",
        target_bir_lowering=False,
        debug=False,
        enable_asserts=False,
        num_devices=N_CORES,
    )
    x = nc.dram_tensor("x", [BL, C, H, W], f32, kind="ExternalInput").ap()
    d = nc.dram_tensor("d", [1, BL], i32, kind="ExternalInput").ap()
    w1t = nc.dram_tensor("w1t", [C, M96], f32, kind="ExternalInput").ap()
    w2t = nc.dram_tensor("w2t", [M96, C], f32, kind="ExternalInput").ap()
    out = nc.dram_tensor("out", [BL, C, H, W], f32, kind="ExternalOutput").ap()

    xr = x.rearrange("b c h w -> b c (h w)")
    outr = out.rearrange("b c h w -> b c (h w)")

    with ExitStack() as ctx:
        tc = ctx.enter_context(tile.TileContext(nc))
        const = ctx.enter_context(tc.tile_pool(name="const", bufs=1))
        xpool = ctx.enter_context(tc.tile_pool(name="xp", bufs=8))
        small = ctx.enter_context(tc.tile_pool(name="small", bufs=4))
        ps_h = ctx.enter_context(tc.tile_pool(name="psh", bufs=2, space="PSUM"))
        ps_g = ctx.enter_context(tc.tile_pool(name="psg", bufs=2, space="PSUM"))

        # ---- weights / routing constants (tiny, loaded once) ----
        # w1_sb columns [96j, 96j+96) hold chunk j: lhsT [K=128 c, M=96 (e,hid)]
        w1_sb = const.tile([P, J * M96], f32)
        for j in range(J):
            nc.sync.dma_start(w1_sb[:, j * M96:(j + 1) * M96], w1t[j * P:(j + 1) * P, :])
        w2_sb = const.tile([M96, C], f32)       # lhsT [K=96, M=128] per c-chunk
        nc.sync.dma_start(w2_sb[:], w2t)
        # dataset replicated across 96 partitions (stride-0 DMA read), cast,
        # then mask[32e+k, b] = (dataset[b] == e) built per 32-aligned block
        di_bc = const.tile([M96, BL], i32)
        nc.sync.dma_start(di_bc[:], d.broadcast_to([M96, BL]))
        df_bc = const.tile([M96, BL], f32)
        nc.vector.tensor_copy(df_bc[:], di_bc[:])          # int32 -> f32 cast
        m_sb = const.tile([M96, BL], f32)
        for e in range(NE):
            nc.vector.tensor_scalar(
                m_sb[e * HID:(e + 1) * HID, :], df_bc[e * HID:(e + 1) * HID, :],
                float(e), None, op0=mybir.AluOpType.is_equal,
            )

        # ---- per-sample pipeline ----
        for b in range(BL):
            xt = []
            for j in range(J):
                t = xpool.tile([P, HW], f32, tag="xt")
                nc.sync.dma_start(t[:], xr[b, j * P:(j + 1) * P, :])
                xt.append(t)
            # channel sums (mean * HW); scale folded into the relu below
            ysum = small.tile([P, J], f32, tag="y")
            for j in range(J):
                nc.vector.tensor_reduce(
                    ysum[:, j:j + 1], xt[j][:],
                    axis=mybir.AxisListType.X, op=mybir.AluOpType.add,
                )
            # h for all 3 experts at once: [96, 1]
            h_ps = ps_h.tile([M96, 1], f32, tag="h")
            for j in range(J):
                nc.tensor.matmul(
                    h_ps[:], w1_sb[:, j * M96:(j + 1) * M96], ysum[:, j:j + 1],
                    start=(j == 0), stop=(j == J - 1),
                )
            h_sb = small.tile([M96, 1], f32, tag="hs")
            nc.scalar.activation(h_sb[:], h_ps[:], FT.Relu, scale=1.0 / HW)
            hm_sb = small.tile([M96, 1], f32, tag="hm")
            nc.vector.tensor_mul(hm_sb[:], h_sb[:], m_sb[:, b:b + 1])
            # gate[c] for the selected expert, c-chunk j in column j
            g_ps = ps_g.tile([P, J], f32, tag="g")
            for j in range(J):
                nc.tensor.matmul(
                    g_ps[:, j:j + 1], w2_sb[:, j * P:(j + 1) * P], hm_sb[:],
                    start=True, stop=True,
                )
            g_sb = small.tile([P, J], f32, tag="gs")
            nc.scalar.activation(g_sb[:], g_ps[:], FT.Sigmoid)
            # apply gate in place and store; the store is issued from the
            # scalar engine's own HWDGE ring so a store waiting on its mul
            # never head-of-line-blocks the load ring on SP
            for j in range(J):
                nc.scalar.mul(xt[j][:], xt[j][:], g_sb[:, j:j + 1])
                nc.scalar.dma_start(outr[b, j * P:(j + 1) * P, :], xt[j][:])

    nc.compile()
    _nc_cache = nc
    return nc


def _prep_shared(W1, W2):
    # lhsT layouts: w1t[c, 32e+k] = W1[e, k, c]; w2t[32e+k, c] = W2[e, c, k]
    w1t = np.ascontiguousarray(W1.transpose(2, 0, 1).reshape(C, M96)).astype(np.float32, copy=False)
    w2t = np.ascontiguousarray(W2.transpose(0, 2, 1).reshape(M96, C)).astype(np.float32, copy=False)
    return w1t, w2t


def kernel(x, dataset, W1, W2, _trace=False):
    nc = _build()
    w1t, w2t = _prep_shared(W1, W2)
    in_maps = []
    for c in range(N_CORES):
        sl = slice(c * BL, (c + 1) * BL)
        in_maps.append({
            "x": np.ascontiguousarray(x[sl]),
            "d": np.ascontiguousarray(np.asarray(dataset[sl], dtype=np.int32).reshape(1, BL)),
            "w1t": w1t,
            "w2t": w2t,
        })
    res = bass_utils.run_bass_kernel_spmd(
        nc, in_maps, core_ids=list(range(N_CORES)), trace=_trace,
    )
    out = np.concatenate([r["out"] for r in res.results], axis=0)
    if _trace:
        return out, res
    return out


# revision 9
# speedup vs baseline: 8.7845x; 8.7845x over previous
"""Trainium2 Bass kernel for nn_DALayer (moe_routing, squeeze-excite style).

Computation (per sample b):
    y    = mean(x[b], axis=(H,W))                 # [C]
    h    = relu(W1[dataset[b]] @ y)               # [HID]
    gate = sigmoid(W2[dataset[b]] @ h)            # [C]
    out[b] = x[b] * gate[:, None, None]

Sharding: pure data parallel over batch across 8 NeuronCores (8 samples
per core); expert weights replicated.  Single pass over x: each sample's
x tiles stay resident in SBUF between the mean-reduce and the gate
multiply, so HBM traffic per core is 64 MiB in + 64 MiB out.

Expert routing is done on-device: all three experts' W1 rows are stacked
([96, C]) so one accumulating matmul chain produces h for every expert;
a one-hot mask (built from `dataset` with is_equal compares and a tiny
matmul against a constant block-indicator) zeroes the two unselected
experts' h, and a stacked-W2 matmul then yields the selected expert's
gate directly.
"""

import numpy as np
from contextlib import ExitStack

import concourse.tile as tile
from concourse import bacc, mybir
from concourse import bass_utils

# Problem shapes (hardcoded per contract).
B, C, H, W = 64, 512, 64, 64
HW = H * W                 # 4096 spatial elements
N_CORES = 8
BL = B // N_CORES          # 8 samples per core
NE, HID = 3, 32
M96 = NE * HID             # 96 stacked expert-hidden rows
P = 128                    # SBUF partitions
J = C // P                 # 4 channel chunks of 128

_nc_cache = {}


def _build(passes=1):
    """Build + compile the per-core Bass module (cached).

    passes>1 repeats the whole pipeline (for timing: T(2)-T(1) cancels
    fixed dispatch overhead)."""
    if passes in _nc_cache:
        return _nc_cache[passes]

    f32 = mybir.dt.float32
    i32 = mybir.dt.int32
    FT = mybir.ActivationFunctionType

    nc = bacc.Bacc(
        "TRN2",
        target_bir_lowering=False,
        debug=False,
        enable_asserts=False,
        num_devices=N_CORES,
    )
    x = nc.dram_tensor("x", [BL, C, H, W], f32, kind="ExternalInput").ap()
    d = nc.dram_tensor("d", [1, BL], i32, kind="ExternalInput").ap()
    w1t = nc.dram_tensor("w1t", [C, M96], f32, kind="ExternalInput").ap()
    w2t = nc.dram_tensor("w2t", [M96, C], f32, kind="ExternalInput").ap()
    out = nc.dram_tensor("out", [BL, C, H, W], f32, kind="ExternalOutput").ap()

    xr = x.rearrange("b c h w -> b c (h w)")
    outr = out.rearrange("b c h w -> b c (h w)")

    with ExitStack() as ctx:
        tc = ctx.enter_context(tile.TileContext(nc))
        const = ctx.enter_context(tc.tile_pool(name="const", bufs=1))
        xpool = ctx.enter_context(tc.tile_pool(name="xp", bufs=8))
        small = ctx.enter_context(tc.tile_pool(name="small", bufs=4))
        ps_h = ctx.enter_context(tc.tile_pool(name="psh", bufs=2, space="PSUM"))
        ps_g = ctx.enter_context(tc.tile_pool(name="psg", bufs=2, space="PSUM"))

        # ---- weights / routing constants (tiny, loaded once) ----
        # w1_sb columns [96j, 96j+96) hold chunk j: lhsT [K=128 c, M=96 (e,hid)]
        w1_sb = const.tile([P, J * M96], f32)
        for j in range(J):
            nc.sync.dma_start(w1_sb[:, j * M96:(j + 1) * M96], w1t[j * P:(j + 1) * P, :])
        w2_sb = const.tile([M96, C], f32)       # lhsT [K=96, M=128] per c-chunk
        nc.sync.dma_start(w2_sb[:], w2t)
        # dataset replicated across 96 partitions (stride-0 DMA read), cast,
        # then mask[32e+k, b] = (dataset[b] == e) built per 32-aligned block
        di_bc = const.tile([M96, BL], i32)
        nc.sync.dma_start(di_bc[:], d.broadcast_to([M96, BL]))
        df_bc = const.tile([M96, BL], f32)
        nc.vector.tensor_copy(df_bc[:], di_bc[:])          # int32 -> f32 cast
        m_sb = const.tile([M96, BL], f32)
        for e in range(NE):
            nc.vector.tensor_scalar(
                m_sb[e * HID:(e + 1) * HID, :], df_bc[e * HID:(e + 1) * HID, :],
                float(e), None, op0=mybir.AluOpType.is_equal,
            )

        # ---- per-sample pipeline ----
        for b in [bb for _ in range(passes) for bb in range(BL)]:
            xt = []
            for j in range(J):
                t = xpool.tile([P, HW], f32, tag="xt")
                nc.sync.dma_start(t[:], xr[b, j * P:(j + 1) * P, :])
                xt.append(t)
            # channel sums (mean * HW); scale folded into the relu below
            ysum = small.tile([P, J], f32, tag="y")
            for j in range(J):
                nc.vector.tensor_reduce(
                    ysum[:, j:j + 1], xt[j][:],
                    axis=mybir.AxisListType.X, op=mybir.AluOpType.add,
                )
            # h for all 3 experts at once: [96, 1]
            h_ps = ps_h.tile([M96, 1], f32, tag="h")
            for j in range(J):
                nc.tensor.matmul(
                    h_ps[:], w1_sb[:, j * M96:(j + 1) * M96], ysum[:, j:j + 1],
                    start=(j == 0), stop=(j == J - 1),
                )
            h_sb = small.tile([M96, 1], f32, tag="hs")
            nc.scalar.activation(h_sb[:], h_ps[:], FT.Relu, scale=1.0 / HW)
            hm_sb = small.tile([M96, 1], f32, tag="hm")
            nc.vector.tensor_mul(hm_sb[:], h_sb[:], m_sb[:, b:b + 1])
            # gate[c] for the selected expert, c-chunk j in column j
            g_ps = ps_g.tile([P, J], f32, tag="g")
            for j in range(J):
                nc.tensor.matmul(
                    g_ps[:, j:j + 1], w2_sb[:, j * P:(j + 1) * P], hm_sb[:],
                    start=True, stop=True,
                )
            g_sb = small.tile([P, J], f32, tag="gs")
            nc.scalar.activation(g_sb[:], g_ps[:], FT.Sigmoid)
            # apply gate in place and store; the store is issued from the
            # scalar engine's own HWDGE ring so a store waiting on its mul
            # never head-of-line-blocks the load ring on SP
            for j in range(J):
                nc.scalar.mul(xt[j][:], xt[j][:], g_sb[:, j:j + 1])
                nc.scalar.dma_start(outr[b, j * P:(j + 1) * P, :], xt[j][:])

    nc.compile()
    _nc_cache[passes] = nc
    return nc


def _prep_shared(W1, W2):
    # lhsT layouts: w1t[c, 32e+k] = W1[e, k, c]; w2t[32e+k, c] = W2[e, c, k]
    w1t = np.ascontiguousarray(W1.transpose(2, 0, 1).reshape(C, M96)).astype(np.float32, copy=False)
    w2t = np.ascontiguousarray(W2.transpose(0, 2, 1).reshape(M96, C)).astype(np.float32, copy=False)
    return w1t, w2t


def kernel(x, dataset, W1, W2, _trace=False):
    nc = _build()
    w1t, w2t = _prep_shared(W1, W2)
    in_maps = []
    for c in range(N_CORES):
        sl = slice(c * BL, (c + 1) * BL)
        in_maps.append({
            "x": np.ascontiguousarray(x[sl]),
            "d": np.ascontiguousarray(np.asarray(dataset[sl], dtype=np.int32).reshape(1, BL)),
            "w1t": w1t,
            "w2t": w2t,
        })
    res = bass_utils.run_bass_kernel_spmd(
        nc, in_maps, core_ids=list(range(N_CORES)), trace=_trace,
    )
    out = np.concatenate([r["out"] for r in res.results], axis=0)
    if _trace:
        return out, res
    return out
